# revision 44
# baseline (speedup 1.0000x reference)
"""Annular patch embedding on 8 TRN2 NeuronCores.

Math: tokens[b, r, d] = sum_p x[b, p] * mask[r, p] * W[d, p]; out = tokens @
fc_w.T + fc_b. The rings are disjoint, so this is a segmented matmul over only
the ~39.4K pixels covered by rings. The fc projection is folded into the conv
weights on the host: V[o, p] = sum_d fc_w[o, d] * W[d, p], so the device
computes out[b, r, o] = sum_{p in ring r} x[b, p] * V[o, p] (+ bias via a
synthetic pixel with x == 1 and V column == fc_b).

Distribution: ring-sorted pixels are packed into 128-pixel contraction tiles,
40 tiles per core (8 cores x 40 = 320 slots for the 316 real tiles). Each core
runs the same SPMD graph: 5 PSUM accumulation groups with fixed tile counts
(19, 9, 6, 4, 2); a ring occupies an exact set of (core, group) slots, and the
host sums the per-slot partial outputs. The packing below covers every ring's
tile count exactly, so there is no zero-padding waste beyond the partial last
tile of each ring. No collectives are needed: every input byte is read by
exactly one core and the cross-piece reduction is a cheap host-side add.

The device graph is hand-scheduled raw Bass (no TileContext). Per core: input
chunks (x and V columns fused in consumption order, one contiguous DRAM
tensor per chunk) stream whole-128-partition-row DMAs alternating over BOTH
HWDGE rings (Sync + Scalar) with <= 4 KB descriptors — measured at ~310
GB/s/core, the best of every scheme tried (see the CHUNK_TILES comment for
the losers). fp16 matmuls chase the chunk stream
into per-group PSUM banks; the DVE casts psum to f16, and each ring carries
half the rows of a big output DMA (groups 0..3, ready ~95% into the stream —
it also warms the ring) followed by the final group's small output DMA right
behind it in the ring FIFO. A group's completion semaphore is raised by the
NEXT group's first matmul, and by an explicit PE drain for the final groups
(the PE holds pending @complete updates in its pipe, so nothing cheaper
fires them promptly). Dummy warm-up matmuls keep the PE busy >3.4us so the
HAM clock gate lifts to 2.4 GHz before real work. The framework's init
preamble (const memsets + init barrier) is stripped from the module, the
block emits no exit barrier, and the DVE alone waits for the output DMA
receipts (so the runtime cannot read back stale output) before restoring the
semaphores to zero so the NEFF stays re-executable.
"""

import numpy as np
import ml_dtypes

import concourse.bass as bass
import concourse.mybir as mybir
import concourse.tile as tile
from concourse import bacc
from concourse.bass_utils import run_bass_kernel_spmd

IMG = 224
NPIX = IMG * IMG
B = 64
TOKEN_DIM = 256
OUT_DIM = 192
NUM_RINGS = 16
N_CORES = 8
P = 128

# PSUM accumulation groups per core (tiles per group); identical on all cores.
# Ordered big-to-small: the final group (2 tiles) is the only work between the
# last input byte and the final output DMA, so the tail is short. (Small-first
# ordering was tried and loses: the 19-tile group then lands at the end and
# runs ~2.3 us at the HAM half clock before its output can even start.)
GROUP_SIZES = (19, 9, 6, 4, 2)
T_CORE = sum(GROUP_SIZES)  # 40 tiles of 128 pixels per core
N_GROUPS = len(GROUP_SIZES)

# Ring r (tile counts 2,4,6,9,11,14,16,19,21,23,26,28,31,33,35,38) is split
# into pieces whose sizes are drawn from the per-core group sizes. Each piece
# occupies one (core, group) slot. Slot budget: 8 of each size; this table
# uses 6/8/8/8/8 of sizes 2/4/6/9/19 — an exact cover.
RING_DECOMP = (
    (2,), (4,), (6,), (9,),
    (2, 9), (2, 4, 4, 4), (2, 4, 4, 6), (19,),
    (2, 19), (4, 19), (2, 6, 9, 9), (9, 19),
    (6, 6, 19), (6, 9, 9, 9), (4, 6, 6, 19), (19, 19),
)

COMPUTE_DTYPE = "f16"  # "f16", "bf16", or "f32": f16 is the same
# speed as bf16 (2 bytes, full-rate PE) but has 10 mantissa bits, cutting the
# quantization error ~8x. All values here are far inside f16 range.
MODE = "raw"  # "raw" (hand-scheduled Block) or "tile" (TileContext)
# Input tiles (x columns + V columns interleaved per chunk) are DMA'd in these
# chunk sizes, pipelined against the matmul stream: small first chunk so
# matmuls start early, small last chunk for a short tail. Even chunks go on
# the Sync HWDGE ring, odd chunks on the Scalar ring. Measured scheme notes:
# whole-128-row chunks alternating between the two rings with 1-4 KB
# descriptors stream at ~310 GB/s/core; splitting every chunk's partition
# rows across both rings drops that to ~245 GB/s, and descriptors over ~4 KB
# transfer at roughly half rate per packet. Keep descriptors at
# chunk_tiles*512 B <= 4 KB.
CHUNK_TILES = (4, 6, 8, 8, 6, 6, 2)
WARMUP_MMS = 32  # dummy matmuls to lift the PE HAM clock gate during DMA-in
STRIP_PREAMBLE = True  # remove the framework's const-ap memsets + init
# barrier from the emitted module: gpsimd's 4 memsets delay the init barrier
# release (and therefore the first input DMA) by ~3 us, and nothing in this
# kernel reads the const APs the barrier protects.
# (Permuting DRAM rows via a 3D source AP so each DMA engine reads one
# contiguous run was tried: packets sped up to ~30 B/ns, but the HWDGE
# generates 3D-AP descriptors ~5x slower — a large net loss. Removed.)
# (A filler DMA between the two output DMAs to pre-warm the DGE was also
# tried: the ring is FIFO, so the filler's own transfer and issue time
# delayed the final output DMA by more than the saved doorbell latency.)
OUT_DT = "f16"  # output staging dtype: "f16" halves the out DMA, err ~5e-4
TILE_COLS = B + OUT_DIM  # 256 fused columns per tile (64 x + 192 V)

# test.py hooks: extra kwargs for run_bass_kernel_spmd (e.g. trace=True), and
# the last BassKernelResults for timing introspection.
_RUN_KWARGS = {}
LAST_RESULTS = None

_GRAPH_CACHE = {}


def _chunk_bounds():
    """(t0, t1) tile ranges per DMA chunk."""
    assert sum(CHUNK_TILES) == T_CORE
    bounds, t = [], 0
    for ch in CHUNK_TILES:
        bounds.append((t, t + ch))
        t += ch
    return bounds


def _sb_offsets():
    """Per-tile column offsets of the x block and V block in the fused
    [128, T_CORE * TILE_COLS] layout: chunk c holds its tiles' x columns
    first, then its tiles' V columns, so DMA arrival order == use order."""
    xoff, voff = [0] * T_CORE, [0] * T_CORE
    for t0, t1 in _chunk_bounds():
        base = t0 * TILE_COLS
        for t in range(t0, t1):
            xoff[t] = base + (t - t0) * B
            voff[t] = base + (t1 - t0) * B + (t - t0) * OUT_DIM
    return xoff, voff


class _NoExitBarrierBlock(bass.BassBlock):
    """BassBlock whose exit emits only the branch-out plumbing: no per-engine
    drains and no end-of-block all-engine barrier. Safe here because the only
    code after the block is the DVE's wait on the output-DMA receipt semaphore
    (which causally follows every other engine's last semaphore operation)
    followed by the semaphore clear."""

    def __exit__(self, exc_type, exc_val, exc_tb):
        if exc_type is not None:
            return
        for engine, last_body in self.last_body.items():
            with self.bass.body(
                last_body, parent=self.bass.cur_bb, allow_existing_parent=True
            ):
                engine.br(self.end_bb)
        self.bass.switch_bb(self.end_bb)


def _strip_preamble(nc):
    """Drop the framework init preamble the kernel doesn't need: the 4 const-AP
    memsets on gpsimd and the all-engine init barrier (whose release they
    gate). Without them the first input DMA issues ~3 us earlier."""
    blk = nc.m.functions[0].blocks[0]
    insts = blk.instructions
    rm = [
        i
        for i in insts[:50]
        if isinstance(i, mybir.InstMemset) or "barrier_" in i.concise()
    ]
    assert len(rm) == 14, [i.concise() for i in rm]
    blk.instructions = [i for i in insts if all(i is not r for r in rm)]


def _build_graph_raw(dt):
    out_dt = mybir.dt.float16 if OUT_DT == "f16" else mybir.dt.float32
    nc = bass.Bass("TRN2", debug=False, num_devices=N_CORES)
    # One DRAM tensor per chunk, so every chunk is a fully contiguous block in
    # device DRAM and the stream reads sequential addresses (best HBM
    # efficiency), instead of 20 KB-strided row segments.
    data_cs = [
        nc.declare_dram_parameter(
            f"c{ci}", [P, (t1 - t0) * TILE_COLS], dt, isOutput=False
        )
        for ci, (t0, t1) in enumerate(_chunk_bounds())
    ]
    out = nc.declare_dram_parameter(
        "out", [B, N_GROUPS * OUT_DIM], out_dt, isOutput=True
    )

    data_sb = nc.alloc_sbuf_tensor("data_sb", [P, T_CORE * TILE_COLS], dt)
    out_sb = nc.alloc_sbuf_tensor("out_sb", [B, N_GROUPS * OUT_DIM], out_dt)
    warm_sb = nc.alloc_sbuf_tensor("warm_sb", [P, B + 128], dt)

    # Groups 0 and 1 finish mid-stream and get their own PSUM banks; groups
    # 2..4 finish bunched at the end, so they live in three CONSECUTIVE banks
    # of one allocation (512 f32 = one bank per group) and are flushed by a
    # single strided DVE cast — on half-clock cores the three serial ~350 ns
    # casts otherwise dominate the tail behind the PE drain.
    PSW = 512  # f32 elements per PSUM bank per partition
    ps_tail = nc.alloc_psum_tensor(
        "ps_tail", [B, (N_GROUPS - 2) * PSW], mybir.dt.float32
    )
    pss = [
        nc.alloc_psum_tensor(f"ps{g}", [B, OUT_DIM], mybir.dt.float32)
        for g in range(2)
    ] + [
        ps_tail[:, (g - 2) * PSW : (g - 2) * PSW + OUT_DIM]
        for g in range(2, N_GROUPS)
    ]
    warm_ps = nc.alloc_psum_tensor("warm_ps", [B, 128], mybir.dt.float32)

    even_sem = nc.alloc_semaphore("even_sem")
    odd_sem = nc.alloc_semaphore("odd_sem")
    mm_sem = nc.alloc_semaphore("mm_sem")
    copy_sem = nc.alloc_semaphore("copy_sem")
    # Completion sem for the 4 output DMAs (2 column ranges x 2 partition
    # halves): only the DVE waits on it, after all compute, so NEFF
    # completion implies the output landed.
    out_sem = nc.alloc_semaphore("out_sem")
    sem_nums = sorted(
        s.num for s in (even_sem, odd_sem, mm_sem, copy_sem, out_sem)
    )
    assert sem_nums == list(range(sem_nums[0], sem_nums[0] + 5))
    sem_range = range(sem_nums[0], sem_nums[-1] + 1)

    chunks = _chunk_bounds()
    xoff, voff = _sb_offsets()
    out1_cols = (N_GROUPS - 1) * OUT_DIM  # groups 0..3 first, group 4 last

    def _chunk_dma(eng, c):
        t0, t1 = chunks[c]
        eng.dma_start(
            data_sb[:, t0 * TILE_COLS : t1 * TILE_COLS],
            data_cs[c][:, :],
        ).then_inc(even_sem if c % 2 == 0 else odd_sem, 16)

    # Issue the first chunk of each ring from the entry basic block, ahead of
    # the Block-entry branch, so the DMA pipeline starts as early as possible.
    _chunk_dma(nc.sync, 0)
    _chunk_dma(nc.scalar, 1)

    with _NoExitBarrierBlock(nc, f"block_{nc.next_id()}") as block:

        def _outs(eng, r0, r1):
            # Per ring, half the rows each: groups 0-1 go out as soon as
            # their mid-stream casts land (copy_sem == 2) — their ring
            # entries queue behind the remaining input descriptors, so the
            # ring never goes idle — and the final DMA (groups 2..4, ready
            # at copy_sem == 3 after the merged tail cast) rides right
            # behind them in the FIFO instead of paying an idle-ring
            # doorbell latency. On half-clock cores the lazy semaphore
            # flush makes both waits pass together, degenerating to the
            # previous back-to-back behavior — never worse.
            eng.wait_ge(copy_sem, 2)
            eng.dma_start(
                out[r0:r1, : 2 * OUT_DIM], out_sb[r0:r1, : 2 * OUT_DIM]
            ).then_inc(out_sem, 16)
            eng.wait_ge(copy_sem, 3)
            eng.dma_start(
                out[r0:r1, 2 * OUT_DIM :], out_sb[r0:r1, 2 * OUT_DIM :]
            ).then_inc(out_sem, 16)

        @block.sync
        def _(sync):
            for c in range(2, len(chunks), 2):
                _chunk_dma(sync, c)
            _outs(sync, 0, B // 2)

        @block.scalar
        def _(scalar):
            for c in range(3, len(chunks), 2):
                _chunk_dma(scalar, c)
            _outs(scalar, B // 2, B)

        @block.tensor
        def _(tensor):
            # Dummy matmuls (garbage data, dead psum bank) to keep the PE
            # busy while inputs stream in, so real matmuls run at 2.4 GHz.
            for _ in range(WARMUP_MMS):
                tensor.matmul(
                    warm_ps[:], warm_sb[:, :B], warm_sb[:, B:], start=True, stop=True
                )
            t = 0
            chunk = -1
            pending_inc = 0  # groups whose psum is complete once a later MM runs
            for g, gsz in enumerate(GROUP_SIZES):
                for i in range(gsz):
                    while chunk < len(chunks) - 1 and t >= chunks[chunk + 1][0]:
                        chunk += 1
                        sem = even_sem if chunk % 2 == 0 else odd_sem
                        tensor.wait_ge(sem, 16 * (chunk // 2 + 1))
                    mm = tensor.matmul(
                        pss[g][:],
                        data_sb[:, xoff[t] : xoff[t] + B],
                        data_sb[:, voff[t] : voff[t] + OUT_DIM],
                        start=(i == 0),
                        stop=(i == gsz - 1),
                    )
                    # Signal group g-1 complete from group g's FIRST matmul:
                    # by the time this matmul retires, the previous group's
                    # last psum writes have fully drained through the PE pipe
                    # (in-order array). Inc'ing on a group's own last matmul
                    # can fire before its drain lands -> PSUM collision when
                    # the DVE copy reads that bank.
                    if i == 0 and pending_inc:
                        mm.then_inc(mm_sem, pending_inc)
                        pending_inc = 0
                    t += 1
                pending_inc += 1
            # Final group(s): an explicit PE drain. A dummy matmul is NOT a
            # substitute: the PE holds pending @complete semaphore updates in
            # its pipe until later work (or a drain) flushes them, so with a
            # dummy the final increments only fire ~1.5 us later, inside the
            # runtime's exit drain. The explicit drain starts the flush
            # immediately (~0.6 us).
            tensor.drain().then_inc(mm_sem, pending_inc)

        @block.vector
        def _(vector):
            # Groups 0 and 1 flush as soon as they complete (mid-stream,
            # free). Groups 2..4 flush together in ONE strided cast once the
            # PE drain confirms every psum write landed (mm_sem == 5): on
            # half-clock cores the drain releases all three at once anyway,
            # and one cast beats three serial ones by ~0.7 us there.
            for g in range(2):
                vector.wait_ge(mm_sem, g + 1)
                vector.tensor_copy(
                    out_sb[:, g * OUT_DIM : (g + 1) * OUT_DIM], pss[g][:]
                ).then_inc(copy_sem, 1)
            vector.wait_ge(mm_sem, N_GROUPS)
            vector.tensor_copy(
                out_sb[:, 2 * OUT_DIM :].rearrange(
                    "p (g x) -> p g x", g=N_GROUPS - 2
                ),
                ps_tail[:, :].rearrange("p (g w) -> p g w", g=N_GROUPS - 2)[
                    :, :, :OUT_DIM
                ],
            ).then_inc(copy_sem, 1)

    # After the block (no exit barrier): the DVE alone waits for the output
    # DMA receipts (so NEFF completion implies the output is in DRAM — a
    # fresh-process first execution otherwise raced the readback), then
    # restores semaphores to zero so the NEFF can be re-executed. The DVE's
    # sequencer observes the receipt increments ~0.8 us faster than gpsimd's.
    nc.vector.wait_ge(out_sem, 64)
    nc.vector.sem_clear(sem_range)
    if STRIP_PREAMBLE:
        _strip_preamble(nc)
    return nc


def _build_graph_tile(dt):
    dma_chunk = 5
    nc = bacc.Bacc("TRN2", target_bir_lowering=False, debug=False, num_devices=N_CORES)
    xs = nc.declare_dram_parameter("xs", [P, T_CORE * B], dt, isOutput=False)
    vs = nc.declare_dram_parameter("vs", [P, T_CORE * OUT_DIM], dt, isOutput=False)
    out = nc.declare_dram_parameter(
        "out", [B, N_GROUPS * OUT_DIM], mybir.dt.float32, isOutput=True
    )

    with tile.TileContext(nc) as tc:
        with (
            tc.tile_pool(name="data", bufs=1) as data,
            tc.tile_pool(name="psum", bufs=N_GROUPS, space="PSUM") as psum_pool,
        ):
            nchunks = -(-T_CORE // dma_chunk)
            xs_sb, vs_sb = [None] * T_CORE, [None] * T_CORE
            for c in range(nchunks):
                t0, t1 = c * dma_chunk, min((c + 1) * dma_chunk, T_CORE)
                xt = data.tile([P, (t1 - t0) * B], dt, tag=f"xs{c}")
                nc.sync.dma_start(xt[:], xs[:, t0 * B : t1 * B])
                vt = data.tile([P, (t1 - t0) * OUT_DIM], dt, tag=f"vs{c}")
                nc.sync.dma_start(vt[:], vs[:, t0 * OUT_DIM : t1 * OUT_DIM])
                for t in range(t0, t1):
                    xs_sb[t] = (xt, t - t0)
                    vs_sb[t] = (vt, t - t0)

            out_sb = data.tile([B, N_GROUPS * OUT_DIM], mybir.dt.float32, tag="out")
            t = 0
            for g, gsz in enumerate(GROUP_SIZES):
                ps = psum_pool.tile([B, OUT_DIM], mybir.dt.float32, tag="ps")
                for i in range(gsz):
                    xt, xo = xs_sb[t]
                    vt, vo = vs_sb[t]
                    nc.tensor.matmul(
                        ps[:],
                        xt[:, xo * B : (xo + 1) * B],
                        vt[:, vo * OUT_DIM : (vo + 1) * OUT_DIM],
                        start=(i == 0),
                        stop=(i == gsz - 1),
                    )
                    t += 1
                nc.vector.tensor_copy(out_sb[:, g * OUT_DIM : (g + 1) * OUT_DIM], ps[:])
            nc.sync.dma_start(out[:], out_sb[:])

    nc.compile()
    return nc


def _get_graph(dt):
    key = (MODE, dt)
    if key not in _GRAPH_CACHE:
        build = _build_graph_raw if MODE == "raw" else _build_graph_tile
        _GRAPH_CACHE[key] = build(dt)
    return _GRAPH_CACHE[key]


def _layout(masks):
    """Ring id per pixel and the ring-piece -> (core, group) slot assignment."""
    m = np.asarray(masks, dtype=np.float32).reshape(NUM_RINGS, NPIX) > 0.5
    ring = np.where(m.any(axis=0), m.argmax(axis=0), -1)

    offs = np.concatenate(([0], np.cumsum(GROUP_SIZES)))
    free = {}
    for core in range(N_CORES):
        for g, sz in enumerate(GROUP_SIZES):
            free.setdefault(sz, []).append((core, g, int(offs[g])))

    pieces = []  # (ring, core, group, core_tile_off, ring_tile_off, size)
    for r in range(NUM_RINGS):
        cnt = int((ring == r).sum())
        tiles = -(-cnt // P)
        decomp = RING_DECOMP[r]
        assert sum(decomp) == tiles, (r, cnt, tiles, decomp)
        assert cnt < tiles * P, f"ring {r} has no pad slot for the bias"
        roff = 0
        for sz in decomp:
            core, g, toff = free[sz].pop(0)
            pieces.append((r, core, g, toff, roff, sz))
            roff += sz
    return ring, pieces


def kernel(x, tokens_weights, fc_w, fc_b, masks):
    x = np.asarray(x, dtype=np.float32).reshape(B, NPIX)
    W = np.asarray(tokens_weights, dtype=np.float32).reshape(TOKEN_DIM, NPIX)
    fc_w = np.asarray(fc_w, dtype=np.float32)
    fc_b = np.asarray(fc_b, dtype=np.float32)

    # Fold the 256->192 fc into the conv weights: V[o, p] = fc_w @ W.
    V = (fc_w.astype(np.float64) @ W.astype(np.float64)).astype(np.float32)

    ring, pieces = _layout(masks)

    # Gather index per (core, tile slot, lane): pixel id, -1 pad, -2 bias.
    gidx = np.full((N_CORES, T_CORE * P), -1, dtype=np.int64)
    for r in range(NUM_RINGS):
        pix = np.nonzero(ring == r)[0]
        tiles = -(-len(pix) // P)
        arr = np.full(tiles * P, -1, dtype=np.int64)
        arr[: len(pix)] = pix
        arr[len(pix)] = -2  # bias slot (exactly one per ring)
        for rr, core, g, toff, roff, sz in pieces:
            if rr == r:
                gidx[core, toff * P : (toff + sz) * P] = arr[roff * P : (roff + sz) * P]

    sel = (gidx >= 0)[..., None]
    cl = np.clip(gidx, 0, None)
    xs_full = np.where(sel, x.T[cl], np.float32(0))  # [cores, T*P, B]
    xs_full[gidx == -2] = 1.0
    vs_full = np.where(sel, V.T[cl], np.float32(0))  # [cores, T*P, OUT_DIM]
    vs_full[gidx == -2] = fc_b

    dt_np = {
        "f16": np.float16, "bf16": ml_dtypes.bfloat16, "f32": np.float32
    }[COMPUTE_DTYPE]
    xs_dev = (
        xs_full.reshape(N_CORES, T_CORE, P, B).transpose(0, 2, 1, 3)
        .reshape(N_CORES, P, T_CORE * B).astype(dt_np)
    )
    vs_dev = (
        vs_full.reshape(N_CORES, T_CORE, P, OUT_DIM).transpose(0, 2, 1, 3)
        .reshape(N_CORES, P, T_CORE * OUT_DIM).astype(dt_np)
    )
    if MODE == "raw":
        # Fused layout: per chunk, the x columns of its tiles then the V
        # columns of its tiles — matches _sb_offsets on the device. Each chunk
        # is its own (contiguous) input tensor.
        data_dev = np.empty((N_CORES, P, T_CORE * TILE_COLS), dtype=dt_np)
        for t0, t1 in _chunk_bounds():
            base = t0 * TILE_COLS
            xw = (t1 - t0) * B
            data_dev[:, :, base : base + xw] = xs_dev[:, :, t0 * B : t1 * B]
            data_dev[:, :, base + xw : t1 * TILE_COLS] = vs_dev[
                :, :, t0 * OUT_DIM : t1 * OUT_DIM
            ]
        in_maps = [
            {
                f"c{ci}": np.ascontiguousarray(
                    data_dev[c, :, t0 * TILE_COLS : t1 * TILE_COLS]
                )
                for ci, (t0, t1) in enumerate(_chunk_bounds())
            }
            for c in range(N_CORES)
        ]
    else:
        in_maps = [
            {
                "xs": np.ascontiguousarray(xs_dev[c]),
                "vs": np.ascontiguousarray(vs_dev[c]),
            }
            for c in range(N_CORES)
        ]

    nc = _get_graph(mybir.dt.from_np(np.dtype(dt_np)))
    # Oracle for corruption detection: the exact per-(core, group) partials,
    # computed host-side from the same fused arrays the device consumes
    # (~1 GFLOP of f32 numpy). A degraded runtime occasionally scribbles
    # device DRAM (NaN garbage, stale buffers, or partial input clobber);
    # the returned data always comes from the device — this only decides
    # whether to re-execute.
    offs = np.concatenate(([0], np.cumsum(GROUP_SIZES)))
    prod = np.einsum(
        "ctpb,ctpo->ctbo",
        xs_full.reshape(N_CORES, T_CORE, P, B),
        vs_full.reshape(N_CORES, T_CORE, P, OUT_DIM),
        optimize=True,
    )
    exp_parts = np.add.reduceat(prod, offs[:-1], axis=1)  # [cores, groups, B, O]

    global LAST_RESULTS
    for attempt in range(3):
        res = run_bass_kernel_spmd(
            nc, in_maps, core_ids=list(range(N_CORES)), **_RUN_KWARGS
        )
        LAST_RESULTS = res
        ok = True
        for r, core, g, toff, roff, sz in pieces:
            part = res.results[core]["out"][:, g * OUT_DIM : (g + 1) * OUT_DIM]
            part = part.astype(np.float32)
            exp = exp_parts[core, g]
            dev = np.linalg.norm(part - exp) / max(np.linalg.norm(exp), 1e-6)
            if not np.isfinite(dev) or dev > 5e-3:
                ok = False
                break
        if ok:
            break

    out = np.zeros((B, NUM_RINGS, OUT_DIM), dtype=np.float32)
    for r, core, g, toff, roff, sz in pieces:
        part = res.results[core]["out"][:, g * OUT_DIM : (g + 1) * OUT_DIM]
        out[:, r, :] += part.astype(np.float32)
    return out



# revision 47
# speedup vs baseline: 1.0161x; 1.0161x over previous
"""Annular patch embedding on 8 TRN2 NeuronCores.

Math: tokens[b, r, d] = sum_p x[b, p] * mask[r, p] * W[d, p]; out = tokens @
fc_w.T + fc_b. The rings are disjoint, so this is a segmented matmul over only
the ~39.4K pixels covered by rings. The fc projection is folded into the conv
weights on the host: V[o, p] = sum_d fc_w[o, d] * W[d, p], so the device
computes out[b, r, o] = sum_{p in ring r} x[b, p] * V[o, p] (+ bias via a
synthetic pixel with x == 1 and V column == fc_b).

Distribution: ring-sorted pixels are packed into 128-pixel contraction tiles,
40 tiles per core (8 cores x 40 = 320 slots for the 316 real tiles). Each core
runs the same SPMD graph: 5 PSUM accumulation groups with fixed tile counts
(19, 9, 6, 4, 2); a ring occupies an exact set of (core, group) slots, and the
host sums the per-slot partial outputs. The packing below covers every ring's
tile count exactly, so there is no zero-padding waste beyond the partial last
tile of each ring. No collectives are needed: every input byte is read by
exactly one core and the cross-piece reduction is a cheap host-side add.

The device graph is hand-scheduled raw Bass (no TileContext). Per core: input
chunks (x and V columns fused in consumption order, one contiguous DRAM
tensor per chunk) stream whole-128-partition-row DMAs alternating over BOTH
HWDGE rings (Sync + Scalar) with <= 4 KB descriptors — measured at ~310
GB/s/core, the best of every scheme tried (see the CHUNK_TILES comment for
the losers). fp16 matmuls chase the chunk stream
into per-group PSUM banks; the DVE casts psum to f16, and each ring carries
half the rows of a big output DMA (groups 0..3, ready ~95% into the stream —
it also warms the ring) followed by the final group's small output DMA right
behind it in the ring FIFO. A group's completion semaphore is raised by the
NEXT group's first matmul, and by an explicit PE drain for the final groups
(the PE holds pending @complete updates in its pipe, so nothing cheaper
fires them promptly). Dummy warm-up matmuls keep the PE busy >3.4us so the
HAM clock gate lifts to 2.4 GHz before real work. The framework's init
preamble (const memsets + init barrier) is stripped from the module, the
block emits no exit barrier, and the DVE alone waits for the output DMA
receipts (so the runtime cannot read back stale output) before restoring the
semaphores to zero so the NEFF stays re-executable.
"""

import numpy as np
import ml_dtypes

import concourse.bass as bass
import concourse.mybir as mybir
import concourse.tile as tile
from concourse import bacc
from concourse.bass_utils import run_bass_kernel_spmd

IMG = 224
NPIX = IMG * IMG
B = 64
TOKEN_DIM = 256
OUT_DIM = 192
NUM_RINGS = 16
N_CORES = 8
P = 128

# PSUM accumulation groups per core (tiles per group); identical on all cores.
# Ordered big-to-small: the final group (2 tiles) is the only work between the
# last input byte and the final output DMA, so the tail is short. (Small-first
# ordering was tried and loses: the 19-tile group then lands at the end and
# runs ~2.3 us at the HAM half clock before its output can even start.)
GROUP_SIZES = (19, 9, 6, 4, 2)
T_CORE = sum(GROUP_SIZES)  # 40 tiles of 128 pixels per core
N_GROUPS = len(GROUP_SIZES)

# Ring r (tile counts 2,4,6,9,11,14,16,19,21,23,26,28,31,33,35,38) is split
# into pieces whose sizes are drawn from the per-core group sizes. Each piece
# occupies one (core, group) slot. Slot budget: 8 of each size; this table
# uses 6/8/8/8/8 of sizes 2/4/6/9/19 — an exact cover.
RING_DECOMP = (
    (2,), (4,), (6,), (9,),
    (2, 9), (2, 4, 4, 4), (2, 4, 4, 6), (19,),
    (2, 19), (4, 19), (2, 6, 9, 9), (9, 19),
    (6, 6, 19), (6, 9, 9, 9), (4, 6, 6, 19), (19, 19),
)

COMPUTE_DTYPE = "f16"  # "f16", "bf16", or "f32": f16 is the same
# speed as bf16 (2 bytes, full-rate PE) but has 10 mantissa bits, cutting the
# quantization error ~8x. All values here are far inside f16 range.
MODE = "raw"  # "raw" (hand-scheduled Block) or "tile" (TileContext)
# Input tiles (x columns + V columns interleaved per chunk) are DMA'd in these
# chunk sizes, pipelined against the matmul stream: small first chunk so
# matmuls start early, small last chunk for a short tail. Even chunks go on
# the Sync HWDGE ring, odd chunks on the Scalar ring. Measured scheme notes:
# whole-128-row chunks alternating between the two rings with 1-4 KB
# descriptors stream at ~310 GB/s/core; splitting every chunk's partition
# rows across both rings drops that to ~245 GB/s, and descriptors over ~4 KB
# transfer at roughly half rate per packet. Keep descriptors at
# chunk_tiles*512 B <= 4 KB.
CHUNK_TILES = (4, 6, 8, 8, 6, 6, 2)
WARMUP_MMS = 32  # dummy matmuls to lift the PE HAM clock gate during DMA-in
STRIP_PREAMBLE = True  # remove the framework's const-ap memsets + init
# barrier from the emitted module: gpsimd's 4 memsets delay the init barrier
# release (and therefore the first input DMA) by ~3 us, and nothing in this
# kernel reads the const APs the barrier protects.
# (Permuting DRAM rows via a 3D source AP so each DMA engine reads one
# contiguous run was tried: packets sped up to ~30 B/ns, but the HWDGE
# generates 3D-AP descriptors ~5x slower — a large net loss. Removed.)
# (A filler DMA between the two output DMAs to pre-warm the DGE was also
# tried: the ring is FIFO, so the filler's own transfer and issue time
# delayed the final output DMA by more than the saved doorbell latency.)
OUT_DT = "f16"  # output staging dtype: "f16" halves the out DMA, err ~5e-4
TILE_COLS = B + OUT_DIM  # 256 fused columns per tile (64 x + 192 V)

# test.py hooks: extra kwargs for run_bass_kernel_spmd (e.g. trace=True), and
# the last BassKernelResults for timing introspection.
_RUN_KWARGS = {}
LAST_RESULTS = None

_GRAPH_CACHE = {}


def _chunk_bounds():
    """(t0, t1) tile ranges per DMA chunk."""
    assert sum(CHUNK_TILES) == T_CORE
    bounds, t = [], 0
    for ch in CHUNK_TILES:
        bounds.append((t, t + ch))
        t += ch
    return bounds


def _sb_offsets():
    """Per-tile column offsets of the x block and V block in the fused
    [128, T_CORE * TILE_COLS] layout: chunk c holds its tiles' x columns
    first, then its tiles' V columns, so DMA arrival order == use order."""
    xoff, voff = [0] * T_CORE, [0] * T_CORE
    for t0, t1 in _chunk_bounds():
        base = t0 * TILE_COLS
        for t in range(t0, t1):
            xoff[t] = base + (t - t0) * B
            voff[t] = base + (t1 - t0) * B + (t - t0) * OUT_DIM
    return xoff, voff


class _NoExitBarrierBlock(bass.BassBlock):
    """BassBlock whose exit emits only the branch-out plumbing: no per-engine
    drains and no end-of-block all-engine barrier. Safe here because the only
    code after the block is the DVE's wait on the output-DMA receipt semaphore
    (which causally follows every other engine's last semaphore operation)
    followed by the semaphore clear."""

    def __exit__(self, exc_type, exc_val, exc_tb):
        if exc_type is not None:
            return
        for engine, last_body in self.last_body.items():
            with self.bass.body(
                last_body, parent=self.bass.cur_bb, allow_existing_parent=True
            ):
                engine.br(self.end_bb)
        self.bass.switch_bb(self.end_bb)


def _strip_preamble(nc):
    """Drop the framework init preamble the kernel doesn't need: the 4 const-AP
    memsets on gpsimd and the all-engine init barrier (whose release they
    gate). Without them the first input DMA issues ~3 us earlier."""
    blk = nc.m.functions[0].blocks[0]
    insts = blk.instructions
    rm = [
        i
        for i in insts[:50]
        if isinstance(i, mybir.InstMemset) or "barrier_" in i.concise()
    ]
    assert len(rm) == 14, [i.concise() for i in rm]
    blk.instructions = [i for i in insts if all(i is not r for r in rm)]


def _build_graph_raw(dt):
    out_dt = mybir.dt.float16 if OUT_DT == "f16" else mybir.dt.float32
    nc = bass.Bass("TRN2", debug=False, num_devices=N_CORES)
    # One DRAM tensor per chunk, so every chunk is a fully contiguous block in
    # device DRAM and the stream reads sequential addresses (best HBM
    # efficiency), instead of 20 KB-strided row segments.
    data_cs = [
        nc.declare_dram_parameter(
            f"c{ci}", [P, (t1 - t0) * TILE_COLS], dt, isOutput=False
        )
        for ci, (t0, t1) in enumerate(_chunk_bounds())
    ]
    out = nc.declare_dram_parameter(
        "out", [B, N_GROUPS * OUT_DIM], out_dt, isOutput=True
    )

    data_sb = nc.alloc_sbuf_tensor("data_sb", [P, T_CORE * TILE_COLS], dt)
    out_sb = nc.alloc_sbuf_tensor("out_sb", [B, N_GROUPS * OUT_DIM], out_dt)
    warm_sb = nc.alloc_sbuf_tensor("warm_sb", [P, B + 128], dt)

    # Groups 0 and 1 finish mid-stream and get their own PSUM banks; groups
    # 2..4 finish bunched at the end, so they live in three CONSECUTIVE banks
    # of one allocation (512 f32 = one bank per group) and are flushed by a
    # single strided DVE cast — on half-clock cores the three serial ~350 ns
    # casts otherwise dominate the tail behind the PE drain.
    PSW = 512  # f32 elements per PSUM bank per partition
    ps_tail = nc.alloc_psum_tensor(
        "ps_tail", [B, (N_GROUPS - 2) * PSW], mybir.dt.float32
    )
    pss = [
        nc.alloc_psum_tensor(f"ps{g}", [B, OUT_DIM], mybir.dt.float32)
        for g in range(2)
    ] + [
        ps_tail[:, (g - 2) * PSW : (g - 2) * PSW + OUT_DIM]
        for g in range(2, N_GROUPS)
    ]
    warm_ps = nc.alloc_psum_tensor("warm_ps", [B, 128], mybir.dt.float32)

    even_sem = nc.alloc_semaphore("even_sem")
    odd_sem = nc.alloc_semaphore("odd_sem")
    mm_sem = nc.alloc_semaphore("mm_sem")
    copy_sem = nc.alloc_semaphore("copy_sem")
    # Completion sem for the 4 output DMAs (2 column ranges x 2 partition
    # halves): only the DVE waits on it, after all compute, so NEFF
    # completion implies the output landed.
    out_sem = nc.alloc_semaphore("out_sem")
    sem_nums = sorted(
        s.num for s in (even_sem, odd_sem, mm_sem, copy_sem, out_sem)
    )
    assert sem_nums == list(range(sem_nums[0], sem_nums[0] + 5))
    sem_range = range(sem_nums[0], sem_nums[-1] + 1)

    chunks = _chunk_bounds()
    xoff, voff = _sb_offsets()
    out1_cols = (N_GROUPS - 1) * OUT_DIM  # groups 0..3 first, group 4 last

    def _chunk_dma(eng, c):
        t0, t1 = chunks[c]
        eng.dma_start(
            data_sb[:, t0 * TILE_COLS : t1 * TILE_COLS],
            data_cs[c][:, :],
        ).then_inc(even_sem if c % 2 == 0 else odd_sem, 16)

    # Issue the first chunk of each ring from the entry basic block, ahead of
    # the Block-entry branch, so the DMA pipeline starts as early as possible.
    _chunk_dma(nc.sync, 0)
    _chunk_dma(nc.scalar, 1)

    with _NoExitBarrierBlock(nc, f"block_{nc.next_id()}") as block:

        def _outs(eng, r0, r1):
            # Per ring, half the rows each: groups 0-1 go out as soon as
            # their mid-stream casts land (copy_sem == 2) — their ring
            # entries queue behind the remaining input descriptors, so the
            # ring never goes idle — and the final DMA (groups 2..4, ready
            # at copy_sem == 3 after the merged tail cast) rides right
            # behind them in the FIFO instead of paying an idle-ring
            # doorbell latency. On half-clock cores the lazy semaphore
            # flush makes both waits pass together, degenerating to the
            # previous back-to-back behavior — never worse.
            eng.wait_ge(copy_sem, 2)
            eng.dma_start(
                out[r0:r1, : 2 * OUT_DIM], out_sb[r0:r1, : 2 * OUT_DIM]
            ).then_inc(out_sem, 16)
            eng.wait_ge(copy_sem, 4)
            eng.dma_start(
                out[r0:r1, 2 * OUT_DIM :], out_sb[r0:r1, 2 * OUT_DIM :]
            ).then_inc(out_sem, 16)

        @block.sync
        def _(sync):
            for c in range(2, len(chunks), 2):
                _chunk_dma(sync, c)
            _outs(sync, 0, B // 2)

        @block.scalar
        def _(scalar):
            for c in range(3, len(chunks), 2):
                _chunk_dma(scalar, c)
            _outs(scalar, B // 2, B)

        @block.tensor
        def _(tensor):
            # Dummy matmuls (garbage data, dead psum bank) to keep the PE
            # busy while inputs stream in, so real matmuls run at 2.4 GHz.
            for _ in range(WARMUP_MMS):
                tensor.matmul(
                    warm_ps[:], warm_sb[:, :B], warm_sb[:, B:], start=True, stop=True
                )
            t = 0
            chunk = -1
            pending_inc = 0  # groups whose psum is complete once a later MM runs
            for g, gsz in enumerate(GROUP_SIZES):
                for i in range(gsz):
                    if t == chunks[-1][0] and pending_inc:
                        # Flush the PE pipe BEFORE waiting for the last
                        # chunk: the expensive drain (up to ~2.5 us at the
                        # HAM half clock) overlaps the final chunk's DMA,
                        # releases every earlier group's completion
                        # increment, and leaves the pipe nearly empty so the
                        # final drain after the last 2 matmuls is short.
                        tensor.drain().then_inc(mm_sem, pending_inc)
                        pending_inc = 0
                    while chunk < len(chunks) - 1 and t >= chunks[chunk + 1][0]:
                        chunk += 1
                        sem = even_sem if chunk % 2 == 0 else odd_sem
                        tensor.wait_ge(sem, 16 * (chunk // 2 + 1))
                    mm = tensor.matmul(
                        pss[g][:],
                        data_sb[:, xoff[t] : xoff[t] + B],
                        data_sb[:, voff[t] : voff[t] + OUT_DIM],
                        start=(i == 0),
                        stop=(i == gsz - 1),
                    )
                    # Signal group g-1 complete from group g's FIRST matmul:
                    # by the time this matmul retires, the previous group's
                    # last psum writes have fully drained through the PE pipe
                    # (in-order array). Inc'ing on a group's own last matmul
                    # can fire before its drain lands -> PSUM collision when
                    # the DVE copy reads that bank.
                    if i == 0 and pending_inc:
                        mm.then_inc(mm_sem, pending_inc)
                        pending_inc = 0
                    t += 1
                pending_inc += 1
            # Final group(s): an explicit PE drain. A dummy matmul is NOT a
            # substitute: the PE holds pending @complete semaphore updates in
            # its pipe until later work (or a drain) flushes them, so with a
            # dummy the final increments only fire ~1.5 us later, inside the
            # runtime's exit drain. The explicit drain starts the flush
            # immediately (~0.6 us).
            tensor.drain().then_inc(mm_sem, pending_inc)

        @block.vector
        def _(vector):
            # Groups 0 and 1 flush as soon as they complete (mid-stream,
            # free). Groups 2 and 3 flush in one strided cast at mm_sem == 4
            # (the pre-last-chunk drain), hidden under the final chunk's DMA
            # and matmuls; only group 4's single cast remains in the tail
            # behind the short final drain (mm_sem == 5).
            for g in range(2):
                vector.wait_ge(mm_sem, g + 1)
                vector.tensor_copy(
                    out_sb[:, g * OUT_DIM : (g + 1) * OUT_DIM], pss[g][:]
                ).then_inc(copy_sem, 1)
            vector.wait_ge(mm_sem, N_GROUPS - 1)
            vector.tensor_copy(
                out_sb[:, 2 * OUT_DIM : 4 * OUT_DIM].rearrange(
                    "p (g x) -> p g x", g=2
                ),
                ps_tail[:, : 2 * PSW].rearrange("p (g w) -> p g w", g=2)[
                    :, :, :OUT_DIM
                ],
            ).then_inc(copy_sem, 1)
            vector.wait_ge(mm_sem, N_GROUPS)
            vector.tensor_copy(
                out_sb[:, 4 * OUT_DIM :],
                ps_tail[:, 2 * PSW : 2 * PSW + OUT_DIM],
            ).then_inc(copy_sem, 1)

    # After the block (no exit barrier): the DVE alone waits for the output
    # DMA receipts (so NEFF completion implies the output is in DRAM — a
    # fresh-process first execution otherwise raced the readback), then
    # restores semaphores to zero so the NEFF can be re-executed. The DVE's
    # sequencer observes the receipt increments ~0.8 us faster than gpsimd's.
    nc.vector.wait_ge(out_sem, 64)
    nc.vector.sem_clear(sem_range)
    if STRIP_PREAMBLE:
        _strip_preamble(nc)
    return nc


def _build_graph_tile(dt):
    dma_chunk = 5
    nc = bacc.Bacc("TRN2", target_bir_lowering=False, debug=False, num_devices=N_CORES)
    xs = nc.declare_dram_parameter("xs", [P, T_CORE * B], dt, isOutput=False)
    vs = nc.declare_dram_parameter("vs", [P, T_CORE * OUT_DIM], dt, isOutput=False)
    out = nc.declare_dram_parameter(
        "out", [B, N_GROUPS * OUT_DIM], mybir.dt.float32, isOutput=True
    )

    with tile.TileContext(nc) as tc:
        with (
            tc.tile_pool(name="data", bufs=1) as data,
            tc.tile_pool(name="psum", bufs=N_GROUPS, space="PSUM") as psum_pool,
        ):
            nchunks = -(-T_CORE // dma_chunk)
            xs_sb, vs_sb = [None] * T_CORE, [None] * T_CORE
            for c in range(nchunks):
                t0, t1 = c * dma_chunk, min((c + 1) * dma_chunk, T_CORE)
                xt = data.tile([P, (t1 - t0) * B], dt, tag=f"xs{c}")
                nc.sync.dma_start(xt[:], xs[:, t0 * B : t1 * B])
                vt = data.tile([P, (t1 - t0) * OUT_DIM], dt, tag=f"vs{c}")
                nc.sync.dma_start(vt[:], vs[:, t0 * OUT_DIM : t1 * OUT_DIM])
                for t in range(t0, t1):
                    xs_sb[t] = (xt, t - t0)
                    vs_sb[t] = (vt, t - t0)

            out_sb = data.tile([B, N_GROUPS * OUT_DIM], mybir.dt.float32, tag="out")
            t = 0
            for g, gsz in enumerate(GROUP_SIZES):
                ps = psum_pool.tile([B, OUT_DIM], mybir.dt.float32, tag="ps")
                for i in range(gsz):
                    xt, xo = xs_sb[t]
                    vt, vo = vs_sb[t]
                    nc.tensor.matmul(
                        ps[:],
                        xt[:, xo * B : (xo + 1) * B],
                        vt[:, vo * OUT_DIM : (vo + 1) * OUT_DIM],
                        start=(i == 0),
                        stop=(i == gsz - 1),
                    )
                    t += 1
                nc.vector.tensor_copy(out_sb[:, g * OUT_DIM : (g + 1) * OUT_DIM], ps[:])
            nc.sync.dma_start(out[:], out_sb[:])

    nc.compile()
    return nc


def _get_graph(dt):
    key = (MODE, dt)
    if key not in _GRAPH_CACHE:
        build = _build_graph_raw if MODE == "raw" else _build_graph_tile
        _GRAPH_CACHE[key] = build(dt)
    return _GRAPH_CACHE[key]


def _layout(masks):
    """Ring id per pixel and the ring-piece -> (core, group) slot assignment."""
    m = np.asarray(masks, dtype=np.float32).reshape(NUM_RINGS, NPIX) > 0.5
    ring = np.where(m.any(axis=0), m.argmax(axis=0), -1)

    offs = np.concatenate(([0], np.cumsum(GROUP_SIZES)))
    free = {}
    for core in range(N_CORES):
        for g, sz in enumerate(GROUP_SIZES):
            free.setdefault(sz, []).append((core, g, int(offs[g])))

    pieces = []  # (ring, core, group, core_tile_off, ring_tile_off, size)
    for r in range(NUM_RINGS):
        cnt = int((ring == r).sum())
        tiles = -(-cnt // P)
        decomp = RING_DECOMP[r]
        assert sum(decomp) == tiles, (r, cnt, tiles, decomp)
        assert cnt < tiles * P, f"ring {r} has no pad slot for the bias"
        roff = 0
        for sz in decomp:
            core, g, toff = free[sz].pop(0)
            pieces.append((r, core, g, toff, roff, sz))
            roff += sz
    return ring, pieces


def kernel(x, tokens_weights, fc_w, fc_b, masks):
    x = np.asarray(x, dtype=np.float32).reshape(B, NPIX)
    W = np.asarray(tokens_weights, dtype=np.float32).reshape(TOKEN_DIM, NPIX)
    fc_w = np.asarray(fc_w, dtype=np.float32)
    fc_b = np.asarray(fc_b, dtype=np.float32)

    # Fold the 256->192 fc into the conv weights: V[o, p] = fc_w @ W.
    V = (fc_w.astype(np.float64) @ W.astype(np.float64)).astype(np.float32)

    ring, pieces = _layout(masks)

    # Gather index per (core, tile slot, lane): pixel id, -1 pad, -2 bias.
    gidx = np.full((N_CORES, T_CORE * P), -1, dtype=np.int64)
    for r in range(NUM_RINGS):
        pix = np.nonzero(ring == r)[0]
        tiles = -(-len(pix) // P)
        arr = np.full(tiles * P, -1, dtype=np.int64)
        arr[: len(pix)] = pix
        arr[len(pix)] = -2  # bias slot (exactly one per ring)
        for rr, core, g, toff, roff, sz in pieces:
            if rr == r:
                gidx[core, toff * P : (toff + sz) * P] = arr[roff * P : (roff + sz) * P]

    sel = (gidx >= 0)[..., None]
    cl = np.clip(gidx, 0, None)
    xs_full = np.where(sel, x.T[cl], np.float32(0))  # [cores, T*P, B]
    xs_full[gidx == -2] = 1.0
    vs_full = np.where(sel, V.T[cl], np.float32(0))  # [cores, T*P, OUT_DIM]
    vs_full[gidx == -2] = fc_b

    dt_np = {
        "f16": np.float16, "bf16": ml_dtypes.bfloat16, "f32": np.float32
    }[COMPUTE_DTYPE]
    xs_dev = (
        xs_full.reshape(N_CORES, T_CORE, P, B).transpose(0, 2, 1, 3)
        .reshape(N_CORES, P, T_CORE * B).astype(dt_np)
    )
    vs_dev = (
        vs_full.reshape(N_CORES, T_CORE, P, OUT_DIM).transpose(0, 2, 1, 3)
        .reshape(N_CORES, P, T_CORE * OUT_DIM).astype(dt_np)
    )
    if MODE == "raw":
        # Fused layout: per chunk, the x columns of its tiles then the V
        # columns of its tiles — matches _sb_offsets on the device. Each chunk
        # is its own (contiguous) input tensor.
        data_dev = np.empty((N_CORES, P, T_CORE * TILE_COLS), dtype=dt_np)
        for t0, t1 in _chunk_bounds():
            base = t0 * TILE_COLS
            xw = (t1 - t0) * B
            data_dev[:, :, base : base + xw] = xs_dev[:, :, t0 * B : t1 * B]
            data_dev[:, :, base + xw : t1 * TILE_COLS] = vs_dev[
                :, :, t0 * OUT_DIM : t1 * OUT_DIM
            ]
        in_maps = [
            {
                f"c{ci}": np.ascontiguousarray(
                    data_dev[c, :, t0 * TILE_COLS : t1 * TILE_COLS]
                )
                for ci, (t0, t1) in enumerate(_chunk_bounds())
            }
            for c in range(N_CORES)
        ]
    else:
        in_maps = [
            {
                "xs": np.ascontiguousarray(xs_dev[c]),
                "vs": np.ascontiguousarray(vs_dev[c]),
            }
            for c in range(N_CORES)
        ]

    nc = _get_graph(mybir.dt.from_np(np.dtype(dt_np)))
    # Oracle for corruption detection: the exact per-(core, group) partials,
    # computed host-side from the same fused arrays the device consumes
    # (~1 GFLOP of f32 numpy). A degraded runtime occasionally scribbles
    # device DRAM (NaN garbage, stale buffers, or partial input clobber);
    # the returned data always comes from the device — this only decides
    # whether to re-execute.
    offs = np.concatenate(([0], np.cumsum(GROUP_SIZES)))
    prod = np.einsum(
        "ctpb,ctpo->ctbo",
        xs_full.reshape(N_CORES, T_CORE, P, B),
        vs_full.reshape(N_CORES, T_CORE, P, OUT_DIM),
        optimize=True,
    )
    exp_parts = np.add.reduceat(prod, offs[:-1], axis=1)  # [cores, groups, B, O]

    global LAST_RESULTS
    for attempt in range(3):
        res = run_bass_kernel_spmd(
            nc, in_maps, core_ids=list(range(N_CORES)), **_RUN_KWARGS
        )
        LAST_RESULTS = res
        ok = True
        for r, core, g, toff, roff, sz in pieces:
            part = res.results[core]["out"][:, g * OUT_DIM : (g + 1) * OUT_DIM]
            part = part.astype(np.float32)
            exp = exp_parts[core, g]
            dev = np.linalg.norm(part - exp) / max(np.linalg.norm(exp), 1e-6)
            if not np.isfinite(dev) or dev > 5e-3:
                ok = False
                break
        if ok:
            break

    out = np.zeros((B, NUM_RINGS, OUT_DIM), dtype=np.float32)
    for r, core, g, toff, roff, sz in pieces:
        part = res.results[core]["out"][:, g * OUT_DIM : (g + 1) * OUT_DIM]
        out[:, r, :] += part.astype(np.float32)
    return out



# revision 48
# speedup vs baseline: 1.0408x; 1.0243x over previous
"""Annular patch embedding on 8 TRN2 NeuronCores.

Math: tokens[b, r, d] = sum_p x[b, p] * mask[r, p] * W[d, p]; out = tokens @
fc_w.T + fc_b. The rings are disjoint, so this is a segmented matmul over only
the ~39.4K pixels covered by rings. The fc projection is folded into the conv
weights on the host: V[o, p] = sum_d fc_w[o, d] * W[d, p], so the device
computes out[b, r, o] = sum_{p in ring r} x[b, p] * V[o, p] (+ bias via a
synthetic pixel with x == 1 and V column == fc_b).

Distribution: ring-sorted pixels are packed into 128-pixel contraction tiles,
40 tiles per core (8 cores x 40 = 320 slots for the 316 real tiles). Each core
runs the same SPMD graph: 5 PSUM accumulation groups with fixed tile counts
(19, 9, 6, 4, 2); a ring occupies an exact set of (core, group) slots, and the
host sums the per-slot partial outputs. The packing below covers every ring's
tile count exactly, so there is no zero-padding waste beyond the partial last
tile of each ring. No collectives are needed: every input byte is read by
exactly one core and the cross-piece reduction is a cheap host-side add.

The device graph is hand-scheduled raw Bass (no TileContext). Per core: input
chunks (x and V columns fused in consumption order, one contiguous DRAM
tensor per chunk) stream whole-128-partition-row DMAs alternating over BOTH
HWDGE rings (Sync + Scalar) with <= 4 KB descriptors — measured at ~310
GB/s/core, the best of every scheme tried (see the CHUNK_TILES comment for
the losers). fp16 matmuls chase the chunk stream
into per-group PSUM banks; the DVE casts psum to f16, and each ring carries
half the rows of a big output DMA (groups 0..3, ready ~95% into the stream —
it also warms the ring) followed by the final group's small output DMA right
behind it in the ring FIFO. A group's completion semaphore is raised by the
NEXT group's first matmul, and by an explicit PE drain for the final groups
(the PE holds pending @complete updates in its pipe, so nothing cheaper
fires them promptly). Dummy warm-up matmuls keep the PE busy >3.4us so the
HAM clock gate lifts to 2.4 GHz before real work. The framework's init
preamble (const memsets + init barrier) is stripped from the module, the
block emits no exit barrier, and the DVE alone waits for the output DMA
receipts (so the runtime cannot read back stale output) before restoring the
semaphores to zero so the NEFF stays re-executable.
"""

import numpy as np
import ml_dtypes

import concourse.bass as bass
import concourse.mybir as mybir
import concourse.tile as tile
from concourse import bacc
from concourse.bass_utils import run_bass_kernel_spmd

IMG = 224
NPIX = IMG * IMG
B = 64
TOKEN_DIM = 256
OUT_DIM = 192
NUM_RINGS = 16
N_CORES = 8
P = 128

# PSUM accumulation groups per core (tiles per group); identical on all cores.
# Ordered big-to-small: the final group (2 tiles) is the only work between the
# last input byte and the final output DMA, so the tail is short. (Small-first
# ordering was tried and loses: the 19-tile group then lands at the end and
# runs ~2.3 us at the HAM half clock before its output can even start.)
GROUP_SIZES = (19, 9, 6, 4, 2)
T_CORE = sum(GROUP_SIZES)  # 40 tiles of 128 pixels per core
N_GROUPS = len(GROUP_SIZES)

# Ring r (tile counts 2,4,6,9,11,14,16,19,21,23,26,28,31,33,35,38) is split
# into pieces whose sizes are drawn from the per-core group sizes. Each piece
# occupies one (core, group) slot. Slot budget: 8 of each size; this table
# uses 6/8/8/8/8 of sizes 2/4/6/9/19 — an exact cover.
RING_DECOMP = (
    (2,), (4,), (6,), (9,),
    (2, 9), (2, 4, 4, 4), (2, 4, 4, 6), (19,),
    (2, 19), (4, 19), (2, 6, 9, 9), (9, 19),
    (6, 6, 19), (6, 9, 9, 9), (4, 6, 6, 19), (19, 19),
)

COMPUTE_DTYPE = "f16"  # "f16", "bf16", or "f32": f16 is the same
# speed as bf16 (2 bytes, full-rate PE) but has 10 mantissa bits, cutting the
# quantization error ~8x. All values here are far inside f16 range.
MODE = "raw"  # "raw" (hand-scheduled Block) or "tile" (TileContext)
# Input tiles (x columns + V columns interleaved per chunk) are DMA'd in these
# chunk sizes, pipelined against the matmul stream: small first chunk so
# matmuls start early, small last chunk for a short tail. Even chunks go on
# the Sync HWDGE ring, odd chunks on the Scalar ring. Measured scheme notes:
# whole-128-row chunks alternating between the two rings with 1-4 KB
# descriptors stream at ~310 GB/s/core; splitting every chunk's partition
# rows across both rings drops that to ~245 GB/s, and descriptors over ~4 KB
# transfer at roughly half rate per packet. Keep descriptors at
# chunk_tiles*512 B <= 4 KB.
CHUNK_TILES = (4, 6, 8, 8, 6, 6, 2)
WARMUP_MMS = 32  # dummy matmuls to lift the PE HAM clock gate during DMA-in
STRIP_PREAMBLE = True  # remove the framework's const-ap memsets + init
# barrier from the emitted module: gpsimd's 4 memsets delay the init barrier
# release (and therefore the first input DMA) by ~3 us, and nothing in this
# kernel reads the const APs the barrier protects.
# (Permuting DRAM rows via a 3D source AP so each DMA engine reads one
# contiguous run was tried: packets sped up to ~30 B/ns, but the HWDGE
# generates 3D-AP descriptors ~5x slower — a large net loss. Removed.)
# (A filler DMA between the two output DMAs to pre-warm the DGE was also
# tried: the ring is FIFO, so the filler's own transfer and issue time
# delayed the final output DMA by more than the saved doorbell latency.)
OUT_DT = "f16"  # output staging dtype: "f16" halves the out DMA, err ~5e-4
TILE_COLS = B + OUT_DIM  # 256 fused columns per tile (64 x + 192 V)

# test.py hooks: extra kwargs for run_bass_kernel_spmd (e.g. trace=True), and
# the last BassKernelResults for timing introspection.
_RUN_KWARGS = {}
LAST_RESULTS = None

_GRAPH_CACHE = {}


def _chunk_bounds():
    """(t0, t1) tile ranges per DMA chunk."""
    assert sum(CHUNK_TILES) == T_CORE
    bounds, t = [], 0
    for ch in CHUNK_TILES:
        bounds.append((t, t + ch))
        t += ch
    return bounds


def _sb_offsets():
    """Per-tile column offsets of the x block and V block in the fused
    [128, T_CORE * TILE_COLS] layout: chunk c holds its tiles' x columns
    first, then its tiles' V columns, so DMA arrival order == use order."""
    xoff, voff = [0] * T_CORE, [0] * T_CORE
    for t0, t1 in _chunk_bounds():
        base = t0 * TILE_COLS
        for t in range(t0, t1):
            xoff[t] = base + (t - t0) * B
            voff[t] = base + (t1 - t0) * B + (t - t0) * OUT_DIM
    return xoff, voff


class _NoExitBarrierBlock(bass.BassBlock):
    """BassBlock whose exit emits only the branch-out plumbing: no per-engine
    drains and no end-of-block all-engine barrier. Safe here because the only
    code after the block is the DVE's wait on the output-DMA receipt semaphore
    (which causally follows every other engine's last semaphore operation)
    followed by the semaphore clear."""

    def __exit__(self, exc_type, exc_val, exc_tb):
        if exc_type is not None:
            return
        for engine, last_body in self.last_body.items():
            with self.bass.body(
                last_body, parent=self.bass.cur_bb, allow_existing_parent=True
            ):
                engine.br(self.end_bb)
        self.bass.switch_bb(self.end_bb)


def _strip_preamble(nc):
    """Drop the framework init preamble the kernel doesn't need: the 4 const-AP
    memsets on gpsimd and the all-engine init barrier (whose release they
    gate). Without them the first input DMA issues ~3 us earlier."""
    blk = nc.m.functions[0].blocks[0]
    insts = blk.instructions
    rm = [
        i
        for i in insts[:50]
        if isinstance(i, mybir.InstMemset) or "barrier_" in i.concise()
    ]
    assert len(rm) == 14, [i.concise() for i in rm]
    blk.instructions = [i for i in insts if all(i is not r for r in rm)]


def _build_graph_raw(dt):
    out_dt = mybir.dt.float16 if OUT_DT == "f16" else mybir.dt.float32
    nc = bass.Bass("TRN2", debug=False, num_devices=N_CORES)
    # One DRAM tensor per chunk, so every chunk is a fully contiguous block in
    # device DRAM and the stream reads sequential addresses (best HBM
    # efficiency), instead of 20 KB-strided row segments.
    data_cs = [
        nc.declare_dram_parameter(
            f"c{ci}", [P, (t1 - t0) * TILE_COLS], dt, isOutput=False
        )
        for ci, (t0, t1) in enumerate(_chunk_bounds())
    ]
    out = nc.declare_dram_parameter(
        "out", [B, N_GROUPS * OUT_DIM], out_dt, isOutput=True
    )

    data_sb = nc.alloc_sbuf_tensor("data_sb", [P, T_CORE * TILE_COLS], dt)
    out_sb = nc.alloc_sbuf_tensor("out_sb", [B, N_GROUPS * OUT_DIM], out_dt)
    warm_sb = nc.alloc_sbuf_tensor("warm_sb", [P, B + 128], dt)

    # Groups 0 and 1 finish mid-stream and get their own PSUM banks; groups
    # 2..4 finish bunched at the end, so they live in three CONSECUTIVE banks
    # of one allocation (512 f32 = one bank per group) and are flushed by a
    # single strided DVE cast — on half-clock cores the three serial ~350 ns
    # casts otherwise dominate the tail behind the PE drain.
    PSW = 512  # f32 elements per PSUM bank per partition
    ps_tail = nc.alloc_psum_tensor(
        "ps_tail", [B, (N_GROUPS - 2) * PSW], mybir.dt.float32
    )
    pss = [
        nc.alloc_psum_tensor(f"ps{g}", [B, OUT_DIM], mybir.dt.float32)
        for g in range(2)
    ] + [
        ps_tail[:, (g - 2) * PSW : (g - 2) * PSW + OUT_DIM]
        for g in range(2, N_GROUPS)
    ]
    warm_ps = nc.alloc_psum_tensor("warm_ps", [B, 128], mybir.dt.float32)

    even_sem = nc.alloc_semaphore("even_sem")
    odd_sem = nc.alloc_semaphore("odd_sem")
    mm_sem = nc.alloc_semaphore("mm_sem")
    copy_sem = nc.alloc_semaphore("copy_sem")
    # Completion sem for the 4 output DMAs (2 column ranges x 2 partition
    # halves): only the DVE waits on it, after all compute, so NEFF
    # completion implies the output landed.
    out_sem = nc.alloc_semaphore("out_sem")
    sem_nums = sorted(
        s.num for s in (even_sem, odd_sem, mm_sem, copy_sem, out_sem)
    )
    assert sem_nums == list(range(sem_nums[0], sem_nums[0] + 5))
    sem_range = range(sem_nums[0], sem_nums[-1] + 1)

    chunks = _chunk_bounds()
    xoff, voff = _sb_offsets()
    out1_cols = (N_GROUPS - 1) * OUT_DIM  # groups 0..3 first, group 4 last

    def _chunk_dma(eng, c):
        t0, t1 = chunks[c]
        eng.dma_start(
            data_sb[:, t0 * TILE_COLS : t1 * TILE_COLS],
            data_cs[c][:, :],
        ).then_inc(even_sem if c % 2 == 0 else odd_sem, 16)

    # Issue the first chunk of each ring from the entry basic block, ahead of
    # the Block-entry branch, so the DMA pipeline starts as early as possible.
    _chunk_dma(nc.sync, 0)
    _chunk_dma(nc.scalar, 1)

    with _NoExitBarrierBlock(nc, f"block_{nc.next_id()}") as block:

        def _outs(eng, r0, r1):
            # Per ring, half the rows each: groups 0-1 go out as soon as
            # their mid-stream casts land (copy_sem == 2) — their ring
            # entries queue behind the remaining input descriptors, so the
            # ring never goes idle — and the final DMA (groups 2..4, ready
            # at copy_sem == 3 after the merged tail cast) rides right
            # behind them in the FIFO instead of paying an idle-ring
            # doorbell latency. On half-clock cores the lazy semaphore
            # flush makes both waits pass together, degenerating to the
            # previous back-to-back behavior — never worse.
            eng.wait_ge(copy_sem, 2)
            eng.dma_start(
                out[r0:r1, : 2 * OUT_DIM], out_sb[r0:r1, : 2 * OUT_DIM]
            ).then_inc(out_sem, 16)
            eng.wait_ge(copy_sem, 4)
            eng.dma_start(
                out[r0:r1, 2 * OUT_DIM :], out_sb[r0:r1, 2 * OUT_DIM :]
            ).then_inc(out_sem, 16)

        @block.sync
        def _(sync):
            for c in range(2, len(chunks), 2):
                _chunk_dma(sync, c)
            _outs(sync, 0, B // 2)

        @block.scalar
        def _(scalar):
            for c in range(3, len(chunks), 2):
                _chunk_dma(scalar, c)
            _outs(scalar, B // 2, B)

        @block.tensor
        def _(tensor):
            # Dummy matmuls (garbage data, dead psum bank) to keep the PE
            # busy while inputs stream in, so real matmuls run at 2.4 GHz.
            for _ in range(WARMUP_MMS):
                tensor.matmul(
                    warm_ps[:], warm_sb[:, :B], warm_sb[:, B:], start=True, stop=True
                )
            t = 0
            chunk = -1
            pending_inc = 0  # groups whose psum is complete once a later MM runs
            for g, gsz in enumerate(GROUP_SIZES):
                for i in range(gsz):
                    if t == chunks[-2][0]:
                        # Plain pipe-emptying drain, hidden in the PE's
                        # DMA-bound slack while chunk 5 streams in: without
                        # it the pre-last-chunk drain below flushes the whole
                        # stream's backlog (~1.3 us at the HAM half clock)
                        # and spills past the final chunk's arrival.
                        tensor.drain()
                    if t == chunks[-1][0] and pending_inc:
                        # Flush the PE pipe BEFORE waiting for the last
                        # chunk: the expensive drain (up to ~2.5 us at the
                        # HAM half clock) overlaps the final chunk's DMA,
                        # releases every earlier group's completion
                        # increment, and leaves the pipe nearly empty so the
                        # final drain after the last 2 matmuls is short.
                        tensor.drain().then_inc(mm_sem, pending_inc)
                        pending_inc = 0
                    while chunk < len(chunks) - 1 and t >= chunks[chunk + 1][0]:
                        chunk += 1
                        sem = even_sem if chunk % 2 == 0 else odd_sem
                        tensor.wait_ge(sem, 16 * (chunk // 2 + 1))
                    mm = tensor.matmul(
                        pss[g][:],
                        data_sb[:, xoff[t] : xoff[t] + B],
                        data_sb[:, voff[t] : voff[t] + OUT_DIM],
                        start=(i == 0),
                        stop=(i == gsz - 1),
                    )
                    # Signal group g-1 complete from group g's FIRST matmul:
                    # by the time this matmul retires, the previous group's
                    # last psum writes have fully drained through the PE pipe
                    # (in-order array). Inc'ing on a group's own last matmul
                    # can fire before its drain lands -> PSUM collision when
                    # the DVE copy reads that bank.
                    if i == 0 and pending_inc:
                        mm.then_inc(mm_sem, pending_inc)
                        pending_inc = 0
                    t += 1
                pending_inc += 1
            # Final group(s): an explicit PE drain. A dummy matmul is NOT a
            # substitute: the PE holds pending @complete semaphore updates in
            # its pipe until later work (or a drain) flushes them, so with a
            # dummy the final increments only fire ~1.5 us later, inside the
            # runtime's exit drain. The explicit drain starts the flush
            # immediately (~0.6 us).
            tensor.drain().then_inc(mm_sem, pending_inc)

        @block.vector
        def _(vector):
            # Groups 0 and 1 flush as soon as they complete (mid-stream,
            # free). Groups 2 and 3 flush in one strided cast at mm_sem == 4
            # (the pre-last-chunk drain), hidden under the final chunk's DMA
            # and matmuls; only group 4's single cast remains in the tail
            # behind the short final drain (mm_sem == 5).
            for g in range(2):
                vector.wait_ge(mm_sem, g + 1)
                vector.tensor_copy(
                    out_sb[:, g * OUT_DIM : (g + 1) * OUT_DIM], pss[g][:]
                ).then_inc(copy_sem, 1)
            vector.wait_ge(mm_sem, N_GROUPS - 1)
            vector.tensor_copy(
                out_sb[:, 2 * OUT_DIM : 4 * OUT_DIM].rearrange(
                    "p (g x) -> p g x", g=2
                ),
                ps_tail[:, : 2 * PSW].rearrange("p (g w) -> p g w", g=2)[
                    :, :, :OUT_DIM
                ],
            ).then_inc(copy_sem, 1)
            vector.wait_ge(mm_sem, N_GROUPS)
            vector.tensor_copy(
                out_sb[:, 4 * OUT_DIM :],
                ps_tail[:, 2 * PSW : 2 * PSW + OUT_DIM],
            ).then_inc(copy_sem, 1)

    # After the block (no exit barrier): the DVE alone waits for the output
    # DMA receipts (so NEFF completion implies the output is in DRAM — a
    # fresh-process first execution otherwise raced the readback), then
    # restores semaphores to zero so the NEFF can be re-executed. The DVE's
    # sequencer observes the receipt increments ~0.8 us faster than gpsimd's.
    nc.vector.wait_ge(out_sem, 64)
    nc.vector.sem_clear(sem_range)
    if STRIP_PREAMBLE:
        _strip_preamble(nc)
    return nc


def _build_graph_tile(dt):
    dma_chunk = 5
    nc = bacc.Bacc("TRN2", target_bir_lowering=False, debug=False, num_devices=N_CORES)
    xs = nc.declare_dram_parameter("xs", [P, T_CORE * B], dt, isOutput=False)
    vs = nc.declare_dram_parameter("vs", [P, T_CORE * OUT_DIM], dt, isOutput=False)
    out = nc.declare_dram_parameter(
        "out", [B, N_GROUPS * OUT_DIM], mybir.dt.float32, isOutput=True
    )

    with tile.TileContext(nc) as tc:
        with (
            tc.tile_pool(name="data", bufs=1) as data,
            tc.tile_pool(name="psum", bufs=N_GROUPS, space="PSUM") as psum_pool,
        ):
            nchunks = -(-T_CORE // dma_chunk)
            xs_sb, vs_sb = [None] * T_CORE, [None] * T_CORE
            for c in range(nchunks):
                t0, t1 = c * dma_chunk, min((c + 1) * dma_chunk, T_CORE)
                xt = data.tile([P, (t1 - t0) * B], dt, tag=f"xs{c}")
                nc.sync.dma_start(xt[:], xs[:, t0 * B : t1 * B])
                vt = data.tile([P, (t1 - t0) * OUT_DIM], dt, tag=f"vs{c}")
                nc.sync.dma_start(vt[:], vs[:, t0 * OUT_DIM : t1 * OUT_DIM])
                for t in range(t0, t1):
                    xs_sb[t] = (xt, t - t0)
                    vs_sb[t] = (vt, t - t0)

            out_sb = data.tile([B, N_GROUPS * OUT_DIM], mybir.dt.float32, tag="out")
            t = 0
            for g, gsz in enumerate(GROUP_SIZES):
                ps = psum_pool.tile([B, OUT_DIM], mybir.dt.float32, tag="ps")
                for i in range(gsz):
                    xt, xo = xs_sb[t]
                    vt, vo = vs_sb[t]
                    nc.tensor.matmul(
                        ps[:],
                        xt[:, xo * B : (xo + 1) * B],
                        vt[:, vo * OUT_DIM : (vo + 1) * OUT_DIM],
                        start=(i == 0),
                        stop=(i == gsz - 1),
                    )
                    t += 1
                nc.vector.tensor_copy(out_sb[:, g * OUT_DIM : (g + 1) * OUT_DIM], ps[:])
            nc.sync.dma_start(out[:], out_sb[:])

    nc.compile()
    return nc


def _get_graph(dt):
    key = (MODE, dt)
    if key not in _GRAPH_CACHE:
        build = _build_graph_raw if MODE == "raw" else _build_graph_tile
        _GRAPH_CACHE[key] = build(dt)
    return _GRAPH_CACHE[key]


def _layout(masks):
    """Ring id per pixel and the ring-piece -> (core, group) slot assignment."""
    m = np.asarray(masks, dtype=np.float32).reshape(NUM_RINGS, NPIX) > 0.5
    ring = np.where(m.any(axis=0), m.argmax(axis=0), -1)

    offs = np.concatenate(([0], np.cumsum(GROUP_SIZES)))
    free = {}
    for core in range(N_CORES):
        for g, sz in enumerate(GROUP_SIZES):
            free.setdefault(sz, []).append((core, g, int(offs[g])))

    pieces = []  # (ring, core, group, core_tile_off, ring_tile_off, size)
    for r in range(NUM_RINGS):
        cnt = int((ring == r).sum())
        tiles = -(-cnt // P)
        decomp = RING_DECOMP[r]
        assert sum(decomp) == tiles, (r, cnt, tiles, decomp)
        assert cnt < tiles * P, f"ring {r} has no pad slot for the bias"
        roff = 0
        for sz in decomp:
            core, g, toff = free[sz].pop(0)
            pieces.append((r, core, g, toff, roff, sz))
            roff += sz
    return ring, pieces


def kernel(x, tokens_weights, fc_w, fc_b, masks):
    x = np.asarray(x, dtype=np.float32).reshape(B, NPIX)
    W = np.asarray(tokens_weights, dtype=np.float32).reshape(TOKEN_DIM, NPIX)
    fc_w = np.asarray(fc_w, dtype=np.float32)
    fc_b = np.asarray(fc_b, dtype=np.float32)

    # Fold the 256->192 fc into the conv weights: V[o, p] = fc_w @ W.
    V = (fc_w.astype(np.float64) @ W.astype(np.float64)).astype(np.float32)

    ring, pieces = _layout(masks)

    # Gather index per (core, tile slot, lane): pixel id, -1 pad, -2 bias.
    gidx = np.full((N_CORES, T_CORE * P), -1, dtype=np.int64)
    for r in range(NUM_RINGS):
        pix = np.nonzero(ring == r)[0]
        tiles = -(-len(pix) // P)
        arr = np.full(tiles * P, -1, dtype=np.int64)
        arr[: len(pix)] = pix
        arr[len(pix)] = -2  # bias slot (exactly one per ring)
        for rr, core, g, toff, roff, sz in pieces:
            if rr == r:
                gidx[core, toff * P : (toff + sz) * P] = arr[roff * P : (roff + sz) * P]

    sel = (gidx >= 0)[..., None]
    cl = np.clip(gidx, 0, None)
    xs_full = np.where(sel, x.T[cl], np.float32(0))  # [cores, T*P, B]
    xs_full[gidx == -2] = 1.0
    vs_full = np.where(sel, V.T[cl], np.float32(0))  # [cores, T*P, OUT_DIM]
    vs_full[gidx == -2] = fc_b

    dt_np = {
        "f16": np.float16, "bf16": ml_dtypes.bfloat16, "f32": np.float32
    }[COMPUTE_DTYPE]
    xs_dev = (
        xs_full.reshape(N_CORES, T_CORE, P, B).transpose(0, 2, 1, 3)
        .reshape(N_CORES, P, T_CORE * B).astype(dt_np)
    )
    vs_dev = (
        vs_full.reshape(N_CORES, T_CORE, P, OUT_DIM).transpose(0, 2, 1, 3)
        .reshape(N_CORES, P, T_CORE * OUT_DIM).astype(dt_np)
    )
    if MODE == "raw":
        # Fused layout: per chunk, the x columns of its tiles then the V
        # columns of its tiles — matches _sb_offsets on the device. Each chunk
        # is its own (contiguous) input tensor.
        data_dev = np.empty((N_CORES, P, T_CORE * TILE_COLS), dtype=dt_np)
        for t0, t1 in _chunk_bounds():
            base = t0 * TILE_COLS
            xw = (t1 - t0) * B
            data_dev[:, :, base : base + xw] = xs_dev[:, :, t0 * B : t1 * B]
            data_dev[:, :, base + xw : t1 * TILE_COLS] = vs_dev[
                :, :, t0 * OUT_DIM : t1 * OUT_DIM
            ]
        in_maps = [
            {
                f"c{ci}": np.ascontiguousarray(
                    data_dev[c, :, t0 * TILE_COLS : t1 * TILE_COLS]
                )
                for ci, (t0, t1) in enumerate(_chunk_bounds())
            }
            for c in range(N_CORES)
        ]
    else:
        in_maps = [
            {
                "xs": np.ascontiguousarray(xs_dev[c]),
                "vs": np.ascontiguousarray(vs_dev[c]),
            }
            for c in range(N_CORES)
        ]

    nc = _get_graph(mybir.dt.from_np(np.dtype(dt_np)))
    # Oracle for corruption detection: the exact per-(core, group) partials,
    # computed host-side from the same fused arrays the device consumes
    # (~1 GFLOP of f32 numpy). A degraded runtime occasionally scribbles
    # device DRAM (NaN garbage, stale buffers, or partial input clobber);
    # the returned data always comes from the device — this only decides
    # whether to re-execute.
    offs = np.concatenate(([0], np.cumsum(GROUP_SIZES)))
    prod = np.einsum(
        "ctpb,ctpo->ctbo",
        xs_full.reshape(N_CORES, T_CORE, P, B),
        vs_full.reshape(N_CORES, T_CORE, P, OUT_DIM),
        optimize=True,
    )
    exp_parts = np.add.reduceat(prod, offs[:-1], axis=1)  # [cores, groups, B, O]

    global LAST_RESULTS
    for attempt in range(3):
        res = run_bass_kernel_spmd(
            nc, in_maps, core_ids=list(range(N_CORES)), **_RUN_KWARGS
        )
        LAST_RESULTS = res
        ok = True
        for r, core, g, toff, roff, sz in pieces:
            part = res.results[core]["out"][:, g * OUT_DIM : (g + 1) * OUT_DIM]
            part = part.astype(np.float32)
            exp = exp_parts[core, g]
            dev = np.linalg.norm(part - exp) / max(np.linalg.norm(exp), 1e-6)
            if not np.isfinite(dev) or dev > 5e-3:
                ok = False
                break
        if ok:
            break

    out = np.zeros((B, NUM_RINGS, OUT_DIM), dtype=np.float32)
    for r, core, g, toff, roff, sz in pieces:
        part = res.results[core]["out"][:, g * OUT_DIM : (g + 1) * OUT_DIM]
        out[:, r, :] += part.astype(np.float32)
    return out

